# revision 1
# baseline (speedup 1.0000x reference)
"""BalanceLoss (BCE + OHEM top-k negatives) on 8 trn2 NeuronCores.

Strategy
--------
Data-parallel: the 32x1x640x640 inputs are flattened and split into 8 equal
shards (one per core).  Each core computes four partial sums over its shard:

    sw = sum(gt * mask)                      (positive count)
    sn = sum((1 - gt) * mask)                (negative count)
    sa = sum(gt * mask * ln(pred))           (-pos_loss_sum)
    sb = sum((1 - gt) * mask * ln(1 - pred)) (-neg_loss_sum over ALL negatives)

On the host the shards are merged.  The OHEM top-k reduces to the full
negative sum whenever k = min(sn, 3*sw) == sn (all negatives kept), which is
the regime for this data distribution; an exact host fallback handles k < sn.

Per-core schedule (all five engines share the work so each stays under the
~55us HBM roofline for the 19.7MB/core the kernel must stream):
  - ScalarE: both Ln passes (ln(1-pred) via the free affine scale=-1/bias=1).
  - GpSimd:  w = gt*mask products (and n = mask-w on alternating tiles).
  - PE:      sum(w) (and sum(n) on those tiles) via accumulating matmuls
             against a ones vector into one PSUM bank.
  - VectorE: the loss products as fused affine_mul_reduce (product +
             free-dim sum in a single instruction), plus n on the other tiles.
  - DMA issue is spread over the SP/Pool/ScalarE sequencers (~1us of
    sequencer occupancy per dma_start would otherwise serialize).
"""

import os
import sys

import numpy as np

# ---------------------------------------------------------------- constants
FULL_SHAPE = (32, 1, 640, 640)
TOT = 32 * 640 * 640          # 13_107_200 elements
N_CORES = 8
PER_CORE = TOT // N_CORES     # 1_638_400
P = 128                       # SBUF partitions
W = PER_CORE // P             # 12_800 free-dim elements per partition
NT = 16                       # compute tiles per core
F = W // NT                   # 800 free-dim elements per tile
DMA_GROUP = 2                 # one [P, F*DMA_GROUP] load feeds 2 compute tiles
IO_BUFS = 4
TMP_BUFS = 4
GP_N_TILES = tuple(range(1, 16, 2))  # tiles whose n-chain runs on GpSimd+PE
MMCHUNK = 512                 # PSUM bank width for the PE reductions
NEG_RATIO = 3.0
EPS = 1e-6

_CONCOURSE_PATHS = ("/opt/trn_rl_repo", "/root/.axon_site/_ro/trn_rl_repo")


def _ensure_concourse():
    try:
        import concourse.bass  # noqa: F401
    except ImportError:
        for p in _CONCOURSE_PATHS:
            if os.path.isdir(p) and p not in sys.path:
                sys.path.insert(0, p)
        import concourse.bass  # noqa: F401


_NC_CACHE = {}


def _build_nc(reps=1):
    """Build the per-core Bass program (same program on every core).

    reps > 1 unrolls the whole tile loop `reps` times inside one NEFF for
    benchmarking; accumulators are rewritten per rep so results are
    unchanged."""
    if reps in _NC_CACHE:
        return _NC_CACHE[reps]
    _ensure_concourse()
    import concourse.bacc as bacc
    import concourse.bass as bass
    import concourse.mybir as mybir
    import concourse.tile as tile

    f32 = mybir.dt.float32
    Act = mybir.ActivationFunctionType
    Alu = mybir.AluOpType

    nc = bacc.Bacc(None, target_bir_lowering=False)
    predD = nc.declare_dram_parameter("pred", [P, W], f32, isOutput=False)
    gtD = nc.declare_dram_parameter("gt", [P, W], f32, isOutput=False)
    maskD = nc.declare_dram_parameter("mask", [P, W], f32, isOutput=False)
    # stats columns: [0:NT]=sum(n) per AMR tile (0 on GP_N_TILES),
    # [NT:2NT]=sum(w*l1), [2NT:3NT]=sum(n*l2), [3NT]=sum(w) from PE,
    # [3NT+1]=sum(n) from PE (partition 0 only for the last two).
    outD = nc.declare_dram_parameter("stats", [P, 3 * NT + 2], f32, isOutput=True)

    n_w_mms = NT * ((F + MMCHUNK - 1) // MMCHUNK)
    n_n_mms = len(GP_N_TILES) * ((F + MMCHUNK - 1) // MMCHUNK)

    with tile.TileContext(nc) as tc:
        with (
            tc.tile_pool(name="io", bufs=IO_BUFS) as io_pool,
            tc.tile_pool(name="tmp", bufs=TMP_BUFS) as tmp_pool,
            tc.tile_pool(name="accp", bufs=1) as acc_pool,
            tc.tile_pool(name="ps", bufs=1, space="PSUM") as ps_pool,
        ):
            acc = acc_pool.tile([P, 3 * NT + 2], f32)
            nc.vector.memset(acc[:], 0.0)
            ones = acc_pool.tile([P, 1], f32)
            nc.gpsimd.memset(ones[:], 1.0)
            psum = ps_pool.tile([1, 2 * MMCHUNK], f32)
            FD = F * DMA_GROUP
            for rep in range(reps):
                wm = nm = 0
                gt_g = mask_g = pred_g = None
                for t in range(NT):
                    if t % DMA_GROUP == 0:
                        sl = slice(t * F, t * F + FD)
                        # spread the first loads over three sequencers so the
                        # pipeline fills as early as possible
                        if t == 0:
                            e_pred, e_gt, e_mask = nc.gpsimd, nc.sync, nc.scalar
                        else:
                            e_pred, e_gt, e_mask = nc.sync, nc.sync, nc.gpsimd
                        gt_g = io_pool.tile([P, FD], f32, tag="gt")
                        e_gt.dma_start(gt_g[:], gtD[:, sl])
                        mask_g = io_pool.tile([P, FD], f32, tag="mask")
                        e_mask.dma_start(mask_g[:], maskD[:, sl])
                        pred_g = io_pool.tile([P, FD], f32, tag="pred")
                        e_pred.dma_start(pred_g[:], predD[:, sl])
                    h = (t % DMA_GROUP) * F
                    gt_t = gt_g[:, h : h + F]
                    mask_t = mask_g[:, h : h + F]
                    pred_t = pred_g[:, h : h + F]

                    l1 = tmp_pool.tile([P, F], f32, tag="l1")
                    nc.scalar.activation(l1[:], pred_t, Act.Ln)
                    l2 = tmp_pool.tile([P, F], f32, tag="l2")
                    nc.scalar.activation(l2[:], pred_t, Act.Ln,
                                         bias=1.0, scale=-1.0)

                    w = tmp_pool.tile([P, F], f32, tag="w")
                    nc.gpsimd.tensor_tensor(w[:], gt_t, mask_t, Alu.mult)
                    for c in range(0, F, MMCHUNK):
                        cw = min(MMCHUNK, F - c)
                        nc.tensor.matmul(
                            psum[0:1, 0:cw], ones[:, 0:1], w[:, c : c + cw],
                            start=(wm == 0), stop=(wm == n_w_mms - 1),
                            skip_group_check=True,
                        )
                        wm += 1
                    n = tmp_pool.tile([P, F], f32, tag="n")
                    if t in GP_N_TILES:
                        nc.gpsimd.tensor_tensor(n[:], mask_t, w[:], Alu.subtract)
                        for c in range(0, F, MMCHUNK):
                            cw = min(MMCHUNK, F - c)
                            nc.tensor.matmul(
                                psum[0:1, MMCHUNK : MMCHUNK + cw], ones[:, 0:1],
                                n[:, c : c + cw],
                                start=(nm == 0), stop=(nm == n_n_mms - 1),
                                skip_group_check=True,
                            )
                            nm += 1
                    else:
                        nc.vector.affine_mul_reduce(
                            out=n[:], accum_out=acc[:, t : t + 1],
                            in0=gt_t, in1=mask_t, scale=-1.0, bias=1.0,
                        )
                    ja = tmp_pool.tile([P, F], f32, tag="junk")
                    nc.vector.affine_mul_reduce(
                        out=ja[:], accum_out=acc[:, NT + t : NT + t + 1],
                        in0=w[:], in1=l1[:], scale=1.0, bias=0.0,
                    )
                    jb = tmp_pool.tile([P, F], f32, tag="junk")
                    nc.vector.affine_mul_reduce(
                        out=jb[:], accum_out=acc[:, 2 * NT + t : 2 * NT + t + 1],
                        in0=n[:], in1=l2[:], scale=1.0, bias=0.0,
                    )
                # fold the PSUM accumulators into two acc columns (partition
                # 0) on ScalarE (reads PSUM directly; keeps VectorE's drain
                # path short)
                jf = tmp_pool.tile([1, MMCHUNK], f32, tag="jfold")
                nc.scalar.activation(jf[0:1, :], psum[0:1, 0:MMCHUNK], Act.Copy,
                                     accum_out=acc[0:1, 3 * NT : 3 * NT + 1])
                jf2 = tmp_pool.tile([1, MMCHUNK], f32, tag="jfold")
                nc.scalar.activation(jf2[0:1, :], psum[0:1, MMCHUNK : 2 * MMCHUNK],
                                     Act.Copy,
                                     accum_out=acc[0:1, 3 * NT + 1 : 3 * NT + 2])
            nc.sync.dma_start(outD[:], acc[:])
    nc.finalize()

    _NC_CACHE[reps] = nc
    return nc


def _final_scalar(sw, sn, sa, sb, pred=None, gt=None, mask=None):
    """Host-side merge of the global sums into the balance loss."""
    pos_count = sw
    neg_total = sn
    pos_loss_sum = -sa
    neg_count = min(neg_total, NEG_RATIO * pos_count)
    if neg_count >= neg_total:
        topk_sum = -sb
    else:
        # exact OHEM fallback (never triggered for the shipped distribution):
        # sum of the k hardest negatives, ties split exactly like a sort.
        k = int(neg_count)
        p = np.asarray(pred, dtype=np.float32).ravel()
        g = np.asarray(gt, dtype=np.float32).ravel()
        m = np.asarray(mask, dtype=np.float32).ravel()
        neg_loss = (1.0 - g) * m * (-np.log1p(-p.astype(np.float64)))
        if k <= 0:
            topk_sum = 0.0
        else:
            part = np.partition(neg_loss, neg_loss.size - k)
            topk_sum = float(part[neg_loss.size - k :].sum())
    if neg_count > 0:
        out = (pos_loss_sum + topk_sum) / (pos_count + neg_count + EPS)
    else:
        out = pos_loss_sum / (pos_count + EPS)
    return np.asarray(out, dtype=np.float32).reshape(())


def run_device(pred, gt, mask, trace=False, reps=1, **run_kwargs):
    """Shard, run the Bass kernel on 8 cores, return (sums, raw results)."""
    _ensure_concourse()
    from concourse.bass_utils import run_bass_kernel_spmd

    nc = _build_nc(reps)
    shards = []
    for a in (pred, gt, mask):
        arr = np.ascontiguousarray(np.asarray(a, dtype=np.float32)).reshape(
            N_CORES, P, W
        )
        shards.append(arr)
    in_maps = [
        {"pred": shards[0][i], "gt": shards[1][i], "mask": shards[2][i]}
        for i in range(N_CORES)
    ]
    res = run_bass_kernel_spmd(nc, in_maps, list(range(N_CORES)), trace=trace,
                               **run_kwargs)
    stats = np.stack([np.asarray(r["stats"], dtype=np.float64) for r in res.results])
    # stats: [cores, P, 3*NT+2]; sum over cores and partitions
    s = stats.sum(axis=(0, 1))
    sw = s[3 * NT]
    sn = s[0:NT].sum() + s[3 * NT + 1]
    sa = s[NT : 2 * NT].sum()
    sb = s[2 * NT : 3 * NT].sum()
    return (sw, sn, sa, sb), res


def kernel(pred, gt, mask):
    pred = np.asarray(pred, dtype=np.float32)
    gt = np.asarray(gt, dtype=np.float32)
    mask = np.asarray(mask, dtype=np.float32)
    if pred.shape != FULL_SHAPE:
        # defensive pure-host path for non-conforming shapes
        p64 = pred.astype(np.float64)
        sw = float((gt * mask).sum(dtype=np.float64))
        sn = float(((1.0 - gt) * mask).sum(dtype=np.float64))
        sa = float((gt * mask * np.log(p64)).sum())
        sb = float(((1.0 - gt) * mask * np.log1p(-p64)).sum())
        return _final_scalar(sw, sn, sa, sb, pred, gt, mask)
    (sw, sn, sa, sb), _ = run_device(pred, gt, mask)
    return _final_scalar(sw, sn, sa, sb, pred, gt, mask)



# revision 2
# speedup vs baseline: 1.5116x; 1.5116x over previous
"""BalanceLoss (BCE + OHEM top-k negatives) on 8 trn2 NeuronCores — v7.

Math (gt, mask in {0,1}, pred in (0,1)):
    mask * ln(select(gt, pred, 1-pred)) == ln(1 + h*d)   pointwise, with
    d = gt - pred,  h = (1 - 2*gt)*mask   (h*d = -mask*|gt-pred|).
Device sums:  sc = sum ln(1+h*d)             (Act Ln accumulators)
              e1 = sum h = sn - sw           (DVE AMR accumulators)
              sm = sum mask = sn + sw        (PE ones-matmuls into PSUM)
Host: sw = (sm-e1)/2, sn = (sm+e1)/2; OHEM top-k == full negative sum when
min(sn, 3*sw) == sn (true for this distribution; exact host fallback kept).

Scheduling: fully explicit per-engine instruction streams (EMIT list).
gt/mask tiles 1-7 arrive as fp8 casting DMAs on the gpsimd queue ({0,1}
exact, 1-byte transfer charge); tile 0 f32 via the Act queue; pred f32
via SP. DMAs occupy their issuing engine for the transfer in this cost
model, so bytes and compute are budgeted jointly per engine.
"""

import os
import sys

import numpy as np

FULL_SHAPE = (32, 1, 640, 640)
TOT = 32 * 640 * 640
N_CORES = 8
PER_CORE = TOT // N_CORES     # 1_638_400
P = 128
W = PER_CORE // P             # 12_800
NT = 8
F = W // NT                   # 1600

# EMIT: ordered instruction stream; engines execute their own subsequences
# in this order. Ops:
#   ("sp", tens, c0, c1)    SP-queue f32 DMA
#   ("act", tens, c0, c1)   Act-queue f32 DMA
#   ("pool", tens, c0, c1)  Pool-queue fp8 casting DMA
#   ("d", t, eng) ("q", t, eng)  tensor_tensor on "dve"/"pool"
#   ("h", t)                AMR on DVE
#   ("ln", t)               Act Ln + accum; also emits PE matmuls for t
EMIT = [
    ("pool", "gt", 1600, 3200), ("pool", "mask", 1600, 3200),
    ("sp", "pred", 1600, 3200), ("sp", "pred", 3200, 4800),
    ("sp", "pred", 4800, 6400), ("sp", "pred", 6400, 8000),
    ("sp", "pred", 0, 1600), ("sp", "pred", 8000, 9600),
    ("sp", "pred", 9600, 11200), ("sp", "pred", 11200, 12800),
    ("act", "mask", 0, 1600), ("act", "gt", 0, 1600),
    ("h", 1600, 3200, 1),
    ("pool", "gt", 3200, 6400), ("pool", "mask", 3200, 6400),
    ("d", 1600, 3200, "pool"),
    ("q", 1600, 3200, "dve"), ("mm", 1600, 3200),
    ("h", 3200, 4800, 2),
    ("pool", "gt", 6400, 9600), ("pool", "mask", 6400, 9600),
    ("d", 3200, 4800, "pool"),
    ("ln", 1600, 3200, 1),
    ("h", 4800, 6400, 3),
    ("q", 3200, 4800, "pool"), ("mm", 3200, 4800),
    ("ln", 3200, 4800, 2),
    ("pool", "gt", 9600, 11200), ("pool", "mask", 9600, 11200),
    ("d", 4800, 6400, "pool"),
    ("q", 4800, 6400, "dve"), ("mm", 4800, 6400),
    ("h", 6400, 8000, 4),
    ("pool", "gt", 11200, 12800), ("pool", "mask", 11200, 12800),
    ("ln", 4800, 6400, 3),
    ("d", 6400, 8000, "pool"),
    ("h", 0, 1600, 0),
    ("q", 6400, 8000, "pool"), ("mm", 6400, 8000),
    ("ln", 6400, 8000, 4),
    ("d", 0, 1600, "pool"),
    ("q", 0, 1600, "dve"), ("mm", 0, 1600),
    ("h", 8000, 9600, 5),
    ("d", 8000, 9600, "pool"),
    ("ln", 0, 1600, 0),
    ("h", 9600, 11200, 6), ("h", 11200, 12800, 7),
    ("q", 8000, 9600, "pool"), ("mm", 8000, 9600),
    ("ln", 8000, 9600, 5),
    ("d", 9600, 11200, "pool"),
    ("q", 9600, 11200, "dve"), ("mm", 9600, 11200),
    ("ln", 9600, 11200, 6),
    ("d", 11200, 12800, "pool"),
    ("q", 11200, 12800, "dve"), ("mm", 11200, 12800),
    ("ln", 11200, 12800, 7),
]

MMCHUNK = 320
NEG_RATIO = 3.0
EPS = 1e-6
F8_TILES = (1, 2, 3, 4, 5, 6, 7)

_CONCOURSE_PATHS = ("/opt/trn_rl_repo", "/root/.axon_site/_ro/trn_rl_repo")


def _ensure_concourse():
    try:
        import concourse.bass  # noqa: F401
    except ImportError:
        for p in _CONCOURSE_PATHS:
            if os.path.isdir(p) and p not in sys.path:
                sys.path.insert(0, p)
        import concourse.bass  # noqa: F401


_NC_CACHE = {}


def _build_nc(reps=1):
    if reps in _NC_CACHE:
        return _NC_CACHE[reps]
    _ensure_concourse()
    import concourse.bacc as bacc
    import concourse.mybir as mybir
    import concourse.tile as tile

    f32 = mybir.dt.float32
    f8 = mybir.dt.float8e4
    bf16 = mybir.dt.bfloat16
    ActF = mybir.ActivationFunctionType
    Alu = mybir.AluOpType

    nc = bacc.Bacc(None, target_bir_lowering=False)
    predD = nc.declare_dram_parameter("pred", [P, W], f32, isOutput=False)
    gtD = nc.declare_dram_parameter("gt", [P, W], f32, isOutput=False)
    maskD = nc.declare_dram_parameter("mask", [P, W], f32, isOutput=False)
    outD = nc.declare_dram_parameter("stats", [P, 2 * NT + 1], f32, isOutput=True)
    msumD = nc.declare_dram_parameter("msum", [1, MMCHUNK], f32, isOutput=True)
    dram = {"pred": predD, "gt": gtD, "mask": maskD}
    qeng = {"sp": "sync", "act": "scalar", "pool": "gpsimd"}

    n_mms = W // MMCHUNK

    with tile.TileContext(nc) as tc:
        with (
            tc.tile_pool(name="io", bufs=1) as io_pool,
            tc.tile_pool(name="tmp", bufs=3) as tmp_pool,
            tc.tile_pool(name="accp", bufs=1) as acc_pool,
            tc.tile_pool(name="ps", bufs=1, space="PSUM") as ps_pool,
        ):
            acc_h = acc_pool.tile([P, NT], f32)
            nc.vector.memset(acc_h[:], 0.0)
            acc_ln = acc_pool.tile([P, NT + 1], f32)
            nc.vector.memset(acc_ln[:], 0.0)
            ones_f = acc_pool.tile([P, 1], f32)
            nc.gpsimd.memset(ones_f[:], 1.0)
            ones_8 = acc_pool.tile([P, 1], f8)
            nc.gpsimd.memset(ones_8[:], 1.0)
            psum = ps_pool.tile([1, MMCHUNK], f32)

            for rep in range(reps):
                views = {"pred": [], "gt": [], "mask": []}
                tiles_d = {}
                tiles_h = {}
                tiles_q = {}
                mmi = [0]

                def cview(tens, lo, hi):
                    for c0, c1, b in views[tens]:
                        if c0 <= lo and hi <= c1:
                            return b[:, lo - c0:hi - c0]
                    raise AssertionError(f"no chunk for {tens} [{lo},{hi})")

                def bview(tbl, lo, hi):
                    for (c0, c1), b in tbl.items():
                        if c0 <= lo and hi <= c1:
                            return b[:, lo - c0:hi - c0]
                    raise AssertionError(f"no tile buf [{lo},{hi})")

                for oi, op in enumerate(EMIT):
                    kind = op[0]
                    if kind in ("sp", "act", "pool"):
                        _, tens, c0, c1 = op
                        dt = f8 if kind == "pool" else f32
                        b = io_pool.tile([P, c1 - c0], dt,
                                         tag=f"io{oi}_{rep}")
                        getattr(nc, qeng[kind]).dma_start(
                            b[:], dram[tens][:, c0:c1])
                        views[tens].append((c0, c1, b))
                    elif kind == "d":
                        _, c0, c1, eng = op
                        d = tmp_pool.tile([P, c1 - c0], f32, tag=f"d{c1-c0}")
                        e = nc.vector if eng == "dve" else nc.gpsimd
                        e.tensor_tensor(d[:], cview("gt", c0, c1),
                                        cview("pred", c0, c1), Alu.subtract)
                        tiles_d[(c0, c1)] = d
                    elif kind == "h":
                        _, c0, c1, col = op
                        h = tmp_pool.tile([P, c1 - c0], bf16, tag=f"h{c1-c0}")
                        nc.vector.affine_mul_reduce(
                            out=h[:], accum_out=acc_h[:, col:col + 1],
                            in0=cview("gt", c0, c1), in1=cview("mask", c0, c1),
                            scale=-2.0, bias=1.0)
                        tiles_h[(c0, c1)] = h
                    elif kind == "q":
                        _, c0, c1, eng = op
                        q = tmp_pool.tile([P, c1 - c0], f32, tag=f"q{c1-c0}")
                        e = nc.vector if eng == "dve" else nc.gpsimd
                        e.tensor_tensor(q[:], bview(tiles_h, c0, c1),
                                        bview(tiles_d, c0, c1), Alu.mult)
                        tiles_q[(c0, c1)] = q
                    elif kind == "ln":
                        _, c0, c1, col = op
                        nc.scalar.activation(bview(tiles_d, c0, c1),
                                             bview(tiles_q, c0, c1),
                                             ActF.Ln, bias=1.0, scale=1.0,
                                             accum_out=acc_ln[:, col:col + 1])
                    elif kind == "mm":
                        _, c0, c1 = op
                        f8r = c0 >= 1600
                        ones = ones_8 if f8r else ones_f
                        for c in range(c0, c1, MMCHUNK):
                            nc.tensor.matmul(
                                psum[0:1, :], ones[:, 0:1],
                                cview("mask", c, c + MMCHUNK),
                                start=(mmi[0] == 0),
                                stop=(mmi[0] == n_mms - 1),
                                skip_group_check=True)
                            mmi[0] += 1
                msb = acc_pool.tile([1, MMCHUNK], f32, tag="msb")
                nc.scalar.activation(msb[0:1, :], psum[0:1, :], ActF.Copy)
            nc.sync.dma_start(outD[:, 0:NT], acc_h[:])
            nc.scalar.dma_start(outD[:, NT:2 * NT + 1], acc_ln[:])
            nc.sync.dma_start(msumD[:], msb[:])
    nc.finalize()

    _NC_CACHE[reps] = nc
    return nc


def _final_scalar(e1, sm, sc, pred=None, gt=None, mask=None):
    """Host merge: e1 = sn - sw, sm = sn + sw, sc = -(pos_loss + neg_loss)."""
    sw = (sm - e1) / 2.0
    sn = (sm + e1) / 2.0
    pos_count = sw
    neg_count = min(sn, NEG_RATIO * pos_count)
    if neg_count >= sn:
        total_loss = -sc
    else:
        # exact OHEM fallback (not triggered for the shipped distribution)
        k = int(neg_count)
        p = np.asarray(pred, dtype=np.float64).ravel()
        g = np.asarray(gt, dtype=np.float64).ravel()
        m = np.asarray(mask, dtype=np.float64).ravel()
        pos_loss_sum = float(-(g * m * np.log(p)).sum())
        neg_loss = (1.0 - g) * m * (-np.log1p(-p))
        if k <= 0:
            topk_sum = 0.0
        else:
            part = np.partition(neg_loss, neg_loss.size - k)
            topk_sum = float(part[neg_loss.size - k:].sum())
        total_loss = pos_loss_sum + topk_sum
        if neg_count <= 0:
            return np.float32(pos_loss_sum / (pos_count + EPS)).reshape(())
    if neg_count > 0:
        out = total_loss / (pos_count + neg_count + EPS)
    else:
        out = total_loss / (pos_count + EPS)
    return np.asarray(out, dtype=np.float32).reshape(())


def run_device(pred, gt, mask, trace=False, reps=1, **run_kwargs):
    _ensure_concourse()
    from concourse.bass_utils import run_bass_kernel_spmd

    nc = _build_nc(reps)
    shards = []
    for a in (pred, gt, mask):
        arr = np.ascontiguousarray(np.asarray(a, dtype=np.float32)).reshape(
            N_CORES, P, W
        )
        shards.append(arr)
    in_maps = [
        {"pred": shards[0][i], "gt": shards[1][i], "mask": shards[2][i]}
        for i in range(N_CORES)
    ]
    res = run_bass_kernel_spmd(nc, in_maps, list(range(N_CORES)), trace=trace,
                               **run_kwargs)
    e1 = sc = sm = 0.0
    for r in res.results:
        stats = np.asarray(r["stats"], dtype=np.float64)
        e1 += stats[:, 0:NT].sum()
        sc += stats[:, NT:2 * NT + 1].sum()
        sm += np.asarray(r["msum"], dtype=np.float64).sum()
    return (e1, sm, sc), res


def kernel(pred, gt, mask):
    pred = np.asarray(pred, dtype=np.float32)
    gt = np.asarray(gt, dtype=np.float32)
    mask = np.asarray(mask, dtype=np.float32)
    if pred.shape != FULL_SHAPE:
        p64 = pred.astype(np.float64)
        g64 = gt.astype(np.float64)
        m64 = mask.astype(np.float64)
        sw = float((g64 * m64).sum())
        sn = float(((1.0 - g64) * m64).sum())
        sc = float((g64 * m64 * np.log(p64)).sum()
                   + ((1.0 - g64) * m64 * np.log1p(-p64)).sum())
        return _final_scalar(sn - sw, sn + sw, sc, pred, gt, mask)
    (e1, sm, sc), _ = run_device(pred, gt, mask)
    return _final_scalar(e1, sm, sc, pred, gt, mask)
